# revision 1
# baseline (speedup 1.0000x reference)
"""GatedBlock kernel: data-parallel over 8 NeuronCores.

Shards the leading N axis of x (200000 rows -> 25000/core), replicates
the small per-irrep weights W0/W1/W2, computes the o3.Linear + gate
activation block on each core, and gathers the full [N, 896] output.
"""
import numpy as np
import jax
import jax.numpy as jnp
from functools import partial

N = 200000
MUL0, MUL1, MUL2 = 256, 128, 64
SCALARS, NGATES = 256, 128
MULH = 64
SILU_NORM = 1.6791
SIGMOID_NORM = 1.8484
NCORES = 8

INV0 = 1.0 / np.sqrt(MUL0)
INV1 = 1.0 / np.sqrt(MUL1)
INV2 = 1.0 / np.sqrt(MUL2)


@partial(jax.pmap, in_axes=(0, None, None, None))
def _block(x, W0, W1, W2):
    n = x.shape[0]
    x0 = x[:, :MUL0]
    x1 = x[:, MUL0:MUL0 + MUL1 * 3].reshape(n, MUL1, 3)
    x2 = x[:, MUL0 + MUL1 * 3:].reshape(n, MUL2, 5)

    y0 = (x0 @ W0) * INV0
    # einsum over the multiplicity axis; fold the (2l+1) component axis
    # into the row axis so each is a single dense matmul on-device.
    y1 = jnp.einsum('nmc,mk->nkc', x1, W1) * INV1
    y2 = jnp.einsum('nmc,mk->nkc', x2, W2) * INV2

    out_scalars = jax.nn.silu(y0[:, :SCALARS]) * SILU_NORM
    g = jax.nn.sigmoid(y0[:, SCALARS:]) * SIGMOID_NORM
    g1 = g[:, :MULH, None]
    g2 = g[:, MULH:, None]
    return jnp.concatenate(
        [out_scalars,
         (y1 * g1).reshape(n, MULH * 3),
         (y2 * g2).reshape(n, MULH * 5)],
        axis=1,
    )


def kernel(x, W0, W1, W2):
    x = np.asarray(x, dtype=np.float32)
    W0 = np.asarray(W0, dtype=np.float32)
    W1 = np.asarray(W1, dtype=np.float32)
    W2 = np.asarray(W2, dtype=np.float32)
    n = x.shape[0]
    shard = n // NCORES
    xs = x.reshape(NCORES, shard, x.shape[1])
    out = _block(xs, W0, W1, W2)
    return np.asarray(out).reshape(n, MULH * 5 + MULH * 3 + SCALARS).astype(np.float32)

